# revision 1
# baseline (speedup 1.0000x reference)
"""GCN encoder (2x GCNConv + BatchNorm + PReLU) on 8 Trainium2 NeuronCores.

Full inputs in, full outputs out. Internally:
  - nodes sharded contiguously across 8 cores (12500 real rows + pad -> 12544),
  - v' = dinv * (h @ W) computed locally per core (feat-on-partitions layout),
  - AllGather of the v' table (the "halo exchange" -- random graph means the
    halo is essentially everything),
  - per-edge messages fetched with gpsimd.dma_gather (block-local int16 idxs),
  - segment-sum via gpsimd.dma_scatter_add into an HBM accumulator,
  - conv = dinv * acc (dinv[dst] fold), BN stats via free-dim reduce after a
    PE transpose into [feat, rows] layout, stats AllReduce, fused BN+PReLU
    via two ACT Relu passes + one DVE scalar_tensor_tensor.

norm_e = dinv[src]*dinv[dst] is separable, so no per-edge multiply is needed:
dinv[src] is folded into the gathered table, dinv[dst] into the accumulator
read-out.  BatchNorm makes the conv bias irrelevant (it cancels in x - mean),
so b0/b1 are accepted and ignored.
"""

import math

import numpy as np

import concourse.bass as bass
import concourse.bacc as bacc
import concourse.tile as tile
from concourse import mybir
from concourse import bass_utils
from concourse.masks import make_identity
from concourse.bass_interp import get_hw_module

F32 = mybir.dt.float32
I16 = mybir.dt.int16
EPS = 1e-5
NB = 8  # cores


# ---------------------------------------------------------------- host side


def _wrap16(vals: np.ndarray, cap: int) -> np.ndarray:
    """Pack idx list into the [16, cap//16] layout (j -> [j%16, j//16])."""
    assert vals.shape[0] == cap and cap % 16 == 0
    return np.ascontiguousarray(vals.reshape(cap // 16, 16).T)


def preprocess(x: np.ndarray, edge_index: np.ndarray):
    """Shard nodes, partition edges by (dst core, src block), build idx lists."""
    N = x.shape[0]
    nsh = (N + NB - 1) // NB                     # real rows per shard
    SH = ((nsh + 1 + 127) // 128) * 128          # padded rows (>= nsh+1 zero row)
    zero_row = nsh                               # a guaranteed all-zero table row

    # self-loops are handled analytically on-device (conv += dinv*vloc),
    # so only the real edges go through gather/scatter
    src = edge_index[0]
    dst = edge_index[1]
    deg = np.bincount(dst, minlength=N) + 1  # +1: self-loop
    dinv = (1.0 / np.sqrt(deg.astype(np.float64))).astype(np.float32)

    c_of = dst // nsh                            # owning core of each edge
    b_of = src // nsh                            # source block of each edge
    gloc = (src % nsh).astype(np.int64)
    sloc = (dst % nsh).astype(np.int64)

    counts = np.zeros((NB, NB), np.int64)
    per_cb_g = [[None] * NB for _ in range(NB)]
    per_cb_s = [[None] * NB for _ in range(NB)]
    for c in range(NB):
        mc = c_of == c
        bc = b_of[mc]
        gc = gloc[mc]
        sc = sloc[mc]
        for b in range(NB):
            mb = bc == b
            per_cb_g[c][b] = gc[mb]
            per_cb_s[c][b] = sc[mb]
            counts[c, b] = mb.sum()

    # dma_scatter_add loses updates when duplicate dst rows race within one
    # call, so split each (core, block) edge list into rounds: the k-th edge
    # hitting a given dst goes to round k -> every round is duplicate-free
    # and becomes its own scatter call (calls serialize via the acc WAW dep).
    per_cb_r = [[None] * NB for _ in range(NB)]
    nrounds = np.zeros((NB, NB), np.int64)
    round_sizes = [[None] * NB for _ in range(NB)]
    for c in range(NB):
        for b in range(NB):
            s = per_cb_s[c][b]
            r = np.zeros(len(s), np.int64)
            if len(s):
                order = np.argsort(s, kind="stable")
                ss = s[order]
                occ = np.arange(len(ss)) - np.searchsorted(ss, ss, side="left")
                r[order] = occ
            per_cb_r[c][b] = r
            nrounds[c, b] = int(r.max()) + 1 if len(r) else 0
            round_sizes[c][b] = np.bincount(r) if len(r) else np.zeros(0, np.int64)

    # static per-(block, round) padded sizes = max over cores, 128-multiples
    rounds = []
    for b in range(NB):
        R = int(nrounds[:, b].max())
        sizes = []
        for r in range(R):
            m = max(int(round_sizes[c][b][r]) if r < len(round_sizes[c][b])
                    else 0 for c in range(NB))
            sizes.append(((m + 127) // 128) * 128)
        rounds.append([sz for sz in sizes if sz > 0])
    cap_b = [sum(szs) for szs in rounds]
    tot = sum(cap_b)
    band_off = np.cumsum([0] + cap_b)

    gidx = np.zeros((NB, 128, tot // 16), np.int16)
    sidx = np.zeros((NB, 128, tot // 16), np.int16)
    dinv_cols = np.zeros((NB, 128, SH // 128), np.float32)
    x_sh = np.zeros((NB, SH, x.shape[1]), np.float32)
    for c in range(NB):
        for b in range(NB):
            if cap_b[b] == 0:
                continue
            g = np.full(cap_b[b], zero_row, np.int64)
            # pads must NOT hit row 0: their zero-RMWs race with real row-0
            # updates in the same call. Dump them on the unused zero row.
            s = np.full(cap_b[b], zero_row, np.int64)
            off = 0
            rr = per_cb_r[c][b]
            for r, sz in enumerate(rounds[b]):
                m = rr == r
                n = int(m.sum())
                g[off : off + n] = per_cb_g[c][b][m]
                s[off : off + n] = per_cb_s[c][b][m]
                off += sz
            csl = slice(band_off[b] // 16, band_off[b + 1] // 16)
            gidx[c, :, csl] = np.tile(_wrap16(g.astype(np.int16), cap_b[b]),
                                      (8, 1))
            sidx[c, :, csl] = np.tile(_wrap16(s.astype(np.int16), cap_b[b]),
                                      (8, 1))
        lo = c * nsh
        hi = min(lo + nsh, N)
        d = np.zeros(SH, np.float32)
        d[: hi - lo] = dinv[lo:hi]
        dinv_cols[c] = d.reshape(SH // 128, 128).T
        x_sh[c, : hi - lo] = x[lo:hi]

    return dict(
        N=N, nsh=nsh, SH=SH, rounds=tuple(tuple(r) for r in rounds),
        gidx=gidx, sidx=sidx, dinv_cols=dinv_cols, x_sh=x_sh,
    )


# -------------------------------------------------------------- device side


def build_kernel(N: int, SH: int, rounds, D: int = 128):
    """Build the SPMD Bass program (one program, 8 cores).

    rounds[b] = static (padded, 128-multiple) per-round slot counts for the
    edges sourced from block b; scatter calls are one per round.
    """
    nc = bacc.Bacc("TRN2", target_bir_lowering=False, debug=False,
                   num_devices=NB)
    rg = [list(range(NB))]
    NT = SH // 128               # 128-row tiles per shard
    # 512-col chunks over the SH free dim
    chunks = [(o, min(512, SH - o)) for o in range(0, SH, 512)]
    cap_b = [sum(szs) for szs in rounds]
    tot = sum(cap_b)
    band_off = [0]
    for b in range(NB):
        band_off.append(band_off[-1] + cap_b[b])
    # gather chunks per block: greedy-pack round pieces up to CALL_MAX slots
    # per DMA call (bigger calls overflow SWDGE descriptor capacity on HW);
    # rounds are split freely -- any subset of a duplicate-free round is
    # still duplicate-free
    CALL_MAX = 4096
    gchunks = []
    for b in range(NB):
        pieces = []
        off = 0
        for sz in rounds[b]:
            o = 0
            while o < sz:
                pieces.append((off + o, min(CALL_MAX, sz - o)))
                o += CALL_MAX
            off += sz
        lst, cur = [], []
        for (po, psz) in pieces:
            if cur and (po + psz) - cur[0][0] > CALL_MAX:
                lst.append((cur[0][0], cur))
                cur = []
            cur.append((po, psz))
        if cur:
            lst.append((cur[0][0], cur))
        gchunks.append(lst)
    MT = max((sum(sz for _, sz in rl) for gc in gchunks for _, rl in gc),
             default=128)

    x_in = nc.dram_tensor("x", [SH, D], F32, kind="ExternalInput")
    gidx_in = nc.dram_tensor("gidx", [128, tot // 16], I16, kind="ExternalInput")
    sidx_in = nc.dram_tensor("sidx", [128, tot // 16], I16, kind="ExternalInput")
    dinv_in = nc.dram_tensor("dinv_cols", [128, NT], F32, kind="ExternalInput")
    w_in = [nc.dram_tensor(f"w{l}", [D, D], F32, kind="ExternalInput")
            for l in range(2)]
    gam_in = [nc.dram_tensor(f"gamma{l}", [D, 1], F32, kind="ExternalInput")
              for l in range(2)]
    bet_in = [nc.dram_tensor(f"beta{l}", [D, 1], F32, kind="ExternalInput")
              for l in range(2)]
    a_in = [nc.dram_tensor(f"a{l}", [D, 1], F32, kind="ExternalInput")
            for l in range(2)]
    out_t = nc.dram_tensor("out", [SH, D], F32, kind="ExternalOutput")

    vloc = nc.dram_tensor("vloc", [SH, D], F32)
    vfull = nc.dram_tensor("vfull", [NB * SH, D], F32, addr_space="Shared")
    acc = nc.dram_tensor("acc", [SH, D], F32)
    stats_in = nc.dram_tensor("stats_in", [D, 2], F32)
    stats_out = nc.dram_tensor("stats_out", [D, 2], F32, addr_space="Shared")

    acc_r = acc.ap().rearrange("(t p) f -> t p f", p=128)
    out_r = out_t.ap().rearrange("(t p) f -> t p f", p=128)
    x_r = x_in.ap().rearrange("(t p) f -> t p f", p=128)
    vloc_r = vloc.ap().rearrange("(t p) f -> t p f", p=128)

    with tile.TileContext(nc) as tc:
        with (
            tc.tile_pool(name="pers", bufs=1) as PE_,
            tc.tile_pool(name="act", bufs=1) as PA,
            tc.tile_pool(name="msg", bufs=2) as PM,
            tc.tile_pool(name="work", bufs=3) as PW,
            tc.tile_pool(name="small", bufs=2) as PS,
            tc.tile_pool(name="psA", bufs=2, space="PSUM") as PP,
            tc.tile_pool(name="psT", bufs=4, space="PSUM") as PT,
        ):
            ident = PE_.tile([128, 128], F32, tag="ident")
            make_identity(nc, ident[:])
            gidx_sb = PE_.tile([128, tot // 16], I16, tag="gidx")
            nc.sync.dma_start(gidx_sb[:], gidx_in.ap())
            sidx_sb = PE_.tile([128, tot // 16], I16, tag="sidx")
            nc.sync.dma_start(sidx_sb[:], sidx_in.ap())
            dinv_sb = PE_.tile([128, NT], F32, tag="dinv")
            nc.sync.dma_start(dinv_sb[:], dinv_in.ap())
            w_sb, gam_sb, bet_sb, a_sb = [], [], [], []
            for l in range(2):
                w_sb.append(PE_.tile([128, 128], F32, tag=f"w{l}", name=f"w{l}_sb"))
                nc.sync.dma_start(w_sb[l][:], w_in[l].ap())
                gam_sb.append(PE_.tile([128, 1], F32, tag=f"g{l}", name=f"g{l}_sb"))
                nc.sync.dma_start(gam_sb[l][:], gam_in[l].ap())
                bet_sb.append(PE_.tile([128, 1], F32, tag=f"b{l}", name=f"b{l}_sb"))
                nc.sync.dma_start(bet_sb[l][:], bet_in[l].ap())
                a_sb.append(PE_.tile([128, 1], F32, tag=f"a{l}", name=f"a{l}_sb"))
                nc.sync.dma_start(a_sb[l][:], a_in[l].ap())
            zero_sb = PE_.tile([128, 128], F32, tag="zero")
            nc.vector.memset(zero_sb[:], 0.0)
            eps_sb = PE_.tile([128, 1], F32, tag="eps")
            nc.vector.memset(eps_sb[:], EPS)

            actT = PA.tile([128, SH], F32, tag="actT")  # h_l as [feat, rows]

            # ---- load x, transpose into actT
            for t in range(NT):
                xt = PW.tile([128, 128], F32, tag="xt")
                nc.sync.dma_start(xt[:], x_r[t])
                tp = PT.tile([128, 128], F32, tag="tp")
                nc.tensor.transpose(out=tp[:], in_=xt[:], identity=ident[:])
                nc.vector.tensor_copy(actT[:, 128 * t : 128 * (t + 1)], tp[:])

            for l in range(2):
                # ---- v = W.T-free matmul: vT[fout, rows] = w[fin,fout].T @ actT
                for (o, cw) in chunks:
                    vp = PP.tile([128, 512], F32, tag="vp")
                    nc.tensor.matmul(out=vp[:, :cw], lhsT=w_sb[l][:],
                                     rhs=actT[:, o : o + cw],
                                     start=True, stop=True)
                    vt = PW.tile([128, 512], F32, tag="vt")
                    nc.vector.tensor_copy(vt[:, :cw], vp[:, :cw])
                    # transpose each 128-tile back to [rows, feat], fold dinv[src]
                    for s in range(0, cw, 128):
                        t = (o + s) // 128
                        tp = PT.tile([128, 128], F32, tag="tp")
                        nc.tensor.transpose(out=tp[:], in_=vt[:, s : s + 128],
                                            identity=ident[:])
                        vv = PW.tile([128, 128], F32, tag="vv")
                        nc.vector.tensor_scalar(
                            vv[:], tp[:], dinv_sb[:, t : t + 1], None,
                            op0=mybir.AluOpType.mult)
                        nc.sync.dma_start(vloc_r[t], vv[:])

                # ---- halo exchange: AllGather the v' table
                nc.gpsimd.collective_compute(
                    "AllGather", mybir.AluOpType.bypass, replica_groups=rg,
                    ins=[vloc.ap().opt()], outs=[vfull.ap().opt()])

                # ---- zero accumulator
                for t in range(NT):
                    nc.sync.dma_start(acc_r[t], zero_sb[:])

                # ---- gather messages, scatter-add into acc
                # (one scatter call per duplicate-free round; WAW on acc
                # serializes the RMWs so no same-row races)
                for b in range(NB):
                    for (goff, rlist) in gchunks[b]:
                        gsz = sum(sz for _, sz in rlist)
                        mt = PM.tile([128, MT // 128, 128], F32, tag="mt")
                        mtv = mt[:, : gsz // 128, :]
                        isl = slice((band_off[b] + goff) // 16,
                                    (band_off[b] + goff + gsz) // 16)
                        nc.gpsimd.dma_gather(
                            out_ap=mtv,
                            in_ap=vfull.ap()[b * SH : (b + 1) * SH, :],
                            idxs_ap=gidx_sb[:, isl],
                            num_idxs=gsz, num_idxs_reg=gsz, elem_size=D,
                            single_packet=False)
                        for (roff, rsz) in rlist:
                            ssl = slice((band_off[b] + roff) // 16,
                                        (band_off[b] + roff + rsz) // 16)
                            lo = (roff - goff) // 128
                            nc.gpsimd.dma_scatter_add(
                                out_ap=acc.ap()[:, :],
                                in_ap=mt[:, lo : lo + rsz // 128, :],
                                idxs_ap=sidx_sb[:, ssl],
                                num_idxs=rsz, num_idxs_reg=rsz, elem_size=D,
                                single_packet=False)

                # ---- conv = dinv[dst] * (acc + vloc); transpose into actT
                # (acc + vloc adds the self-loop term dinv[i]^2 * v[i])
                for t in range(NT):
                    at = PW.tile([128, 128], F32, tag="at")
                    nc.sync.dma_start(at[:], acc_r[t])
                    vl = PW.tile([128, 128], F32, tag="vl")
                    nc.sync.dma_start(vl[:], vloc_r[t])
                    sc = PW.tile([128, 128], F32, tag="sc")
                    nc.vector.tensor_tensor(out=sc[:], in0=at[:], in1=vl[:],
                                            op=mybir.AluOpType.add)
                    nc.vector.tensor_scalar(
                        sc[:], sc[:], dinv_sb[:, t : t + 1], None,
                        op0=mybir.AluOpType.mult)
                    tp = PT.tile([128, 128], F32, tag="tp")
                    nc.tensor.transpose(out=tp[:], in_=sc[:], identity=ident[:])
                    nc.vector.tensor_copy(actT[:, 128 * t : 128 * (t + 1)], tp[:])

                # ---- BN stats (biased, over the real N rows; pad rows are 0)
                nk = len(chunks)
                sumc = PS.tile([128, nk], F32, tag="sumc")
                sqc = PS.tile([128, nk], F32, tag="sqc")
                for k, (o, cw) in enumerate(chunks):
                    nc.vector.tensor_reduce(
                        out=sumc[:, k : k + 1], in_=actT[:, o : o + cw],
                        axis=mybir.AxisListType.X, op=mybir.AluOpType.add)
                    sq = PW.tile([128, 512], F32, tag="sq")
                    nc.scalar.activation(
                        out=sq[:, :cw], in_=actT[:, o : o + cw],
                        func=mybir.ActivationFunctionType.Square,
                        bias=zero_sb[:, 0:1],
                        accum_out=sqc[:, k : k + 1])
                stats_sb = PS.tile([128, 2], F32, tag="stats")
                nc.vector.tensor_reduce(out=stats_sb[:, 0:1], in_=sumc[:],
                                        axis=mybir.AxisListType.X,
                                        op=mybir.AluOpType.add)
                nc.vector.tensor_reduce(out=stats_sb[:, 1:2], in_=sqc[:],
                                        axis=mybir.AxisListType.X,
                                        op=mybir.AluOpType.add)
                nc.sync.dma_start(stats_in.ap(), stats_sb[:])
                nc.gpsimd.collective_compute(
                    "AllReduce", mybir.AluOpType.add, replica_groups=rg,
                    ins=[stats_in.ap().opt()], outs=[stats_out.ap().opt()])
                stats2 = PS.tile([128, 2], F32, tag="stats2")
                nc.sync.dma_start(stats2[:], stats_out.ap())

                # ---- BN affine params ([128,1] each)
                mu = PS.tile([128, 1], F32, tag="mu")
                nc.vector.tensor_scalar(mu[:], stats2[:, 0:1], 1.0 / N, None,
                                        op0=mybir.AluOpType.mult)
                e2 = PS.tile([128, 1], F32, tag="e2")
                nc.vector.tensor_scalar(e2[:], stats2[:, 1:2], 1.0 / N, None,
                                        op0=mybir.AluOpType.mult)
                var = PS.tile([128, 1], F32, tag="var")
                nc.vector.scalar_tensor_tensor(
                    out=var[:], in0=mu[:], scalar=-1.0, in1=mu[:],
                    op0=mybir.AluOpType.mult, op1=mybir.AluOpType.mult)
                nc.vector.tensor_tensor(out=var[:], in0=e2[:], in1=var[:],
                                        op=mybir.AluOpType.add)
                sd = PS.tile([128, 1], F32, tag="sd")
                nc.scalar.activation(out=sd[:], in_=var[:],
                                     func=mybir.ActivationFunctionType.Sqrt,
                                     bias=eps_sb[:, 0:1])
                rinv = PS.tile([128, 1], F32, tag="rinv")
                nc.vector.reciprocal(rinv[:], sd[:])
                alpha = PS.tile([128, 1], F32, tag="alpha")
                nc.vector.tensor_tensor(out=alpha[:], in0=gam_sb[l][:],
                                        in1=rinv[:], op=mybir.AluOpType.mult)
                bias_p = PS.tile([128, 1], F32, tag="biasp")
                # bias' = beta - alpha*mu
                nc.vector.scalar_tensor_tensor(
                    out=bias_p[:], in0=alpha[:], scalar=-1.0, in1=mu[:],
                    op0=mybir.AluOpType.mult, op1=mybir.AluOpType.mult)
                nc.vector.tensor_tensor(out=bias_p[:], in0=bet_sb[l][:],
                                        in1=bias_p[:], op=mybir.AluOpType.add)
                nalpha = PS.tile([128, 1], F32, tag="nalpha")
                nc.vector.tensor_scalar(nalpha[:], alpha[:], -1.0, None,
                                        op0=mybir.AluOpType.mult)
                nbias = PS.tile([128, 1], F32, tag="nbias")
                nc.vector.tensor_scalar(nbias[:], bias_p[:], -1.0, None,
                                        op0=mybir.AluOpType.mult)
                na = PS.tile([128, 1], F32, tag="na")
                nc.vector.tensor_scalar(na[:], a_sb[l][:], -1.0, None,
                                        op0=mybir.AluOpType.mult)

                # ---- fused BN + PReLU: y = relu(z) - a*relu(-z), z = alpha*x+bias'
                for (o, cw) in chunks:
                    pos = PW.tile([128, 512], F32, tag="pos")
                    nc.scalar.activation(
                        out=pos[:, :cw], in_=actT[:, o : o + cw],
                        func=mybir.ActivationFunctionType.Relu,
                        bias=bias_p[:, :1], scale=alpha[:, :1])
                    neg = PW.tile([128, 512], F32, tag="neg")
                    nc.scalar.activation(
                        out=neg[:, :cw], in_=actT[:, o : o + cw],
                        func=mybir.ActivationFunctionType.Relu,
                        bias=nbias[:, :1], scale=nalpha[:, :1])
                    # actT = (neg * (-a)) + pos
                    nc.vector.scalar_tensor_tensor(
                        out=actT[:, o : o + cw], in0=neg[:, :cw],
                        scalar=na[:, :1], in1=pos[:, :cw],
                        op0=mybir.AluOpType.mult, op1=mybir.AluOpType.add)

            # ---- write h2 back as [rows, feat]
            for t in range(NT):
                tp = PT.tile([128, 128], F32, tag="tp")
                nc.tensor.transpose(out=tp[:],
                                    in_=actT[:, 128 * t : 128 * (t + 1)],
                                    identity=ident[:])
                ot = PW.tile([128, 128], F32, tag="ot")
                nc.vector.tensor_copy(ot[:], tp[:])
                nc.sync.dma_start(out_r[t], ot[:])

    nc.compile()
    return nc


# ------------------------------------------------------------------- driver

_CACHE: dict = {}


def _get_compiled(key, N, SH, rounds):
    if key not in _CACHE:
        nc = build_kernel(N, SH, rounds)
        nc.m = get_hw_module(nc.m)
        _CACHE[key] = nc
    return _CACHE[key]


def make_in_maps(pre, w0, b0, gamma0, beta0, a0, w1, b1, gamma1, beta1, a1):
    def col(v):
        return np.ascontiguousarray(np.asarray(v, np.float32).reshape(-1, 1))

    def rep(v):
        return np.full((128, 1), np.float32(np.asarray(v).reshape(-1)[0]),
                       np.float32)

    maps = []
    for c in range(NB):
        maps.append({
            "x": pre["x_sh"][c],
            "gidx": pre["gidx"][c],
            "sidx": pre["sidx"][c],
            "dinv_cols": pre["dinv_cols"][c],
            "w0": np.ascontiguousarray(np.asarray(w0, np.float32)),
            "w1": np.ascontiguousarray(np.asarray(w1, np.float32)),
            "gamma0": col(gamma0), "beta0": col(beta0), "a0": rep(a0),
            "gamma1": col(gamma1), "beta1": col(beta1), "a1": rep(a1),
        })
    return maps


def kernel(x, edge_index, w0, b0, gamma0, beta0, a0,
           w1, b1, gamma1, beta1, a1, _trace=False):
    x = np.asarray(x, np.float32)
    edge_index = np.asarray(edge_index, np.int64)
    pre = preprocess(x, edge_index)
    N, nsh, SH = pre["N"], pre["nsh"], pre["SH"]
    key = (N, SH, pre["rounds"])
    nc = _get_compiled(key, N, SH, pre["rounds"])
    in_maps = make_in_maps(pre, w0, b0, gamma0, beta0, a0,
                           w1, b1, gamma1, beta1, a1)
    res = bass_utils.run_bass_kernel_spmd(
        nc, in_maps, core_ids=list(range(NB)), trace=_trace)
    out = np.concatenate([res.results[c]["out"][:nsh] for c in range(NB)],
                         axis=0)[:N]
    if _trace:
        kernel.last_results = res
    return np.ascontiguousarray(out)



# revision 7
# speedup vs baseline: 3.1731x; 3.1731x over previous
"""GCN encoder (2x GCNConv + BatchNorm + PReLU) on 8 Trainium2 NeuronCores.

Full inputs in, full outputs out. v2 design:
  - nodes sharded contiguously across 8 cores (12500 real rows + pad -> 12544),
  - v' = dinv_src * (h @ W) computed locally per core, cast to fp16,
  - AllGather of the fp16 v' table (halo = everything on a random graph),
  - per-edge messages fetched with gpsimd.dma_gather across 4 SWDGE queues
    (descriptor prep parallelizes across queues; this was the v1 bottleneck),
  - segment-sum done ON THE TENSOR ENGINE: edges are sorted by dst tile-pair,
    each 128-edge chunk is multiplied by a DVE-built one-hot selector
    [128 edges x 256 dst slots] and accumulated into a PSUM tile per
    dst-pair -- no dma_scatter_add, no HBM accumulator round trip,
  - conv = dinv_dst * (psum + dinv_dst * v) adds the self-loop analytically,
  - BN stats via free-dim reduce in [feat, rows] layout, stats AllReduce,
    fused BN+PReLU via two ACT Relu passes + one DVE scalar_tensor_tensor.

Stream layout (identical structure on all 8 cores -- SPMD): edges sorted by
(slab, dst-pair) where slab = vfull_row // 32768 (dma_gather idxs are int16,
so each call reads one <=32768-row slab of the AllGathered table). Each
(slab, pair) segment is padded to a 128 multiple with pointers to a known
all-zero table row, and segment sizes are maxed over cores so the compiled
chunk->pair structure is core-independent. Bands (slabs) are padded to the
2048-idx call size, so call k always covers stream chunks [16k, 16k+16).
"""

import numpy as np

import concourse.bass as bass
import concourse.bacc as bacc
import concourse.tile as tile
from concourse import mybir
from concourse import bass_utils
from concourse.masks import make_identity
from concourse.bass_interp import get_hw_module

F32 = mybir.dt.float32
F16 = mybir.dt.float16
I16 = mybir.dt.int16
EPS = 1e-5
NB = 8          # cores
D = 128
SLAB = 32768    # int16 index range per gather call
CALLSZ = 2048   # idxs per dma_gather call (16 chunks)
PAIR = 256      # dst slots per psum accumulation tile


# ---------------------------------------------------------------- host side


def preprocess(x: np.ndarray, edge_index: np.ndarray):
    N = x.shape[0]
    nsh = (N + NB - 1) // NB                     # 12500 real rows per shard
    SH = ((nsh + 1 + 127) // 128) * 128          # 12544 padded rows
    NT = SH // 128                               # 98 tiles
    NP = (SH + PAIR - 1) // PAIR                 # 49 dst pairs
    NSLAB = (NB * SH + SLAB - 1) // SLAB         # 4 slabs over vfull

    src = edge_index[0]
    dst = edge_index[1]
    deg = np.bincount(dst, minlength=N) + 1      # +1: self-loop
    dinv = (1.0 / np.sqrt(deg.astype(np.float64))).astype(np.float32)

    # relative index of a guaranteed all-zero vloc row inside each slab
    zrel = []
    for j in range(NSLAB):
        base = j * SLAB
        zr = None
        for b in range(NB):
            z0 = b * SH + nsh                    # first zero row of block b
            if base <= z0 < min(base + SLAB, NB * SH):
                zr = z0 - base
                break
        assert zr is not None
        zrel.append(zr)

    # per-core edge lists sorted by (slab, dst pair)
    per_core = []
    counts = np.zeros((NB, NSLAB, NP), np.int64)
    for c in range(NB):
        m = (dst // nsh) == c
        gs = src[m]
        sl = (dst[m] - c * nsh).astype(np.int64)
        vrow = (gs // nsh) * SH + (gs % nsh)
        slab = vrow // SLAB
        P = sl // PAIR
        order = np.lexsort((P, slab))
        vrow, sl, slab, P = vrow[order], sl[order], slab[order], P[order]
        per_core.append((vrow, sl, slab, P))
        counts[c] = np.bincount(slab * NP + P,
                                minlength=NSLAB * NP).reshape(NSLAB, NP)

    K = np.ceil(counts.max(axis=0) / 128).astype(np.int64)   # [NSLAB, NP]
    seg_rows = K * 128

    # band-major stream layout; each band padded to a CALLSZ multiple
    band_rows = seg_rows.sum(axis=1)
    band_cap = ((band_rows + CALLSZ - 1) // CALLSZ) * CALLSZ
    band_off = np.concatenate([[0], np.cumsum(band_cap)])
    S = int(band_off[-1])                        # total stream rows
    seg_off = np.zeros((NSLAB, NP), np.int64)
    for j in range(NSLAB):
        seg_off[j] = band_off[j] + np.concatenate(
            [[0], np.cumsum(seg_rows[j])[:-1]])

    # fill per-core index + dst tables
    gidx = np.zeros((NB, 128, S // 16), np.int16)
    dst16 = np.zeros((NB, 128, S // 128), np.float16)
    dinvrow = np.zeros((NB, 128, SH), np.float16)
    dinv_cols = np.zeros((NB, 128, NT), np.float32)
    x_sh = np.zeros((NB, SH, D), np.float32)
    for c in range(NB):
        g = np.zeros(S, np.int64)
        for j in range(NSLAB):
            g[band_off[j] : band_off[j + 1]] = zrel[j]
        dv = np.zeros(S, np.float64)
        vrow, sl, slab, P = per_core[c]
        # slot position for each edge: segment start + rank within segment
        segid = slab * NP + P
        seg_start = seg_off.reshape(-1)[segid]
        # edges are sorted by segid, so rank = index - first index of segid
        first = np.searchsorted(segid, segid, side="left")
        pos = seg_start + (np.arange(len(segid)) - first)
        g[pos] = vrow - slab * SLAB
        dv[pos] = sl % PAIR
        gidx[c] = np.tile(
            np.ascontiguousarray(g.astype(np.int16).reshape(S // 16, 16).T),
            (8, 1))
        dst16[c] = np.ascontiguousarray(
            dv.astype(np.float16).reshape(S // 128, 128).T)

        lo = c * nsh
        hi = min(lo + nsh, N)
        dloc = np.zeros(SH, np.float32)
        dloc[: hi - lo] = dinv[lo:hi]
        dinvrow[c] = np.tile(dloc.astype(np.float16)[None, :], (128, 1))
        dinv_cols[c] = dloc.reshape(NT, 128).T
        x_sh[c, : hi - lo] = x[lo:hi]

    return dict(
        N=N, nsh=nsh, SH=SH, S=S,
        K=tuple(map(tuple, K)), band_off=tuple(int(b) for b in band_off),
        seg_off=tuple(map(tuple, seg_off)),
        gidx=gidx, dst16=dst16, dinvrow=dinvrow, dinv_cols=dinv_cols,
        x_sh=x_sh,
    )


# -------------------------------------------------------------- device side


def build_kernel(N: int, SH: int, S: int, K, band_off, seg_off):
    nc = bacc.Bacc("TRN2", target_bir_lowering=False, debug=False,
                   num_devices=NB, num_swdge_queues=4)
    rg = [list(range(NB))]
    NT = SH // 128
    NP = SH // PAIR
    NSLAB = len(band_off) - 1
    chunks = [(o, min(512, SH - o)) for o in range(0, SH, 512)]

    x_in = nc.dram_tensor("x", [SH, D], F32, kind="ExternalInput")
    gidx_in = nc.dram_tensor("gidx", [128, S // 16], I16,
                             kind="ExternalInput")
    dst_in = nc.dram_tensor("dst16", [128, S // 128], F16,
                            kind="ExternalInput")
    dnr_in = nc.dram_tensor("dinvrow", [128, SH], F16, kind="ExternalInput")
    dinv_in = nc.dram_tensor("dinv_cols", [128, NT], F32,
                             kind="ExternalInput")
    iota_in = nc.dram_tensor("iota256", [128, PAIR], F16,
                             kind="ExternalInput")
    w_in = [nc.dram_tensor(f"w{l}", [D, D], F32, kind="ExternalInput")
            for l in range(2)]
    gam_in = [nc.dram_tensor(f"gamma{l}", [D, 1], F32, kind="ExternalInput")
              for l in range(2)]
    bet_in = [nc.dram_tensor(f"beta{l}", [D, 1], F32, kind="ExternalInput")
              for l in range(2)]
    a_in = [nc.dram_tensor(f"a{l}", [D, 1], F32, kind="ExternalInput")
            for l in range(2)]
    out_t = nc.dram_tensor("out", [SH, D], F32, kind="ExternalOutput")

    vloc = nc.dram_tensor("vloc", [SH, D], F16)
    vfull = nc.dram_tensor("vfull", [NB * SH, D], F16, addr_space="Shared")
    stats_in = nc.dram_tensor("stats_in", [D, 2], F32)
    stats_out = nc.dram_tensor("stats_out", [D, 2], F32, addr_space="Shared")

    out_r = out_t.ap().rearrange("(t p) f -> t p f", p=128)
    x_r = x_in.ap().rearrange("(t p) f -> t p f", p=128)
    vloc_r = vloc.ap().rearrange("(t p) f -> t p f", p=128)

    ncalls = S // CALLSZ
    call_slab = []
    for k in range(ncalls):
        pos = k * CALLSZ
        j = next(jj for jj in range(NSLAB)
                 if band_off[jj] <= pos < band_off[jj + 1])
        call_slab.append(j)

    with tile.TileContext(nc) as tc:
        with (
            tc.tile_pool(name="pers", bufs=1) as PE_,
            tc.tile_pool(name="act", bufs=1) as PA_,
            tc.tile_pool(name="msg", bufs=12) as PM,
            tc.tile_pool(name="sel", bufs=4) as PSL,
            tc.tile_pool(name="work", bufs=3) as PW,
            tc.tile_pool(name="small", bufs=2) as PS,
            tc.tile_pool(name="psP", bufs=3, space="PSUM") as PP,
            tc.tile_pool(name="psV", bufs=1, space="PSUM") as PV,
            tc.tile_pool(name="psA", bufs=1, space="PSUM") as PAP,
            tc.tile_pool(name="psT", bufs=1, space="PSUM") as PT,
        ):
            ident = PE_.tile([128, 128], F32, tag="ident")
            make_identity(nc, ident[:])
            ident16 = PE_.tile([128, 128], F16, tag="ident16")
            nc.vector.tensor_copy(ident16[:], ident[:])
            gidx_sb = PE_.tile([128, S // 16], I16, tag="gidx")
            nc.sync.dma_start(gidx_sb[:], gidx_in.ap())
            dst_sb = PE_.tile([128, S // 128], F16, tag="dst16")
            nc.sync.dma_start(dst_sb[:], dst_in.ap())
            dnr_sb = PE_.tile([128, SH], F16, tag="dinvrow")
            nc.sync.dma_start(dnr_sb[:], dnr_in.ap())
            dinv_sb = PE_.tile([128, NT], F32, tag="dinv")
            nc.sync.dma_start(dinv_sb[:], dinv_in.ap())
            iota_sb = PE_.tile([128, PAIR], F16, tag="iota")
            nc.sync.dma_start(iota_sb[:], iota_in.ap())
            w_sb, gam_sb, bet_sb, a_sb = [], [], [], []
            for l in range(2):
                w_sb.append(PE_.tile([128, 128], F32, tag=f"w{l}",
                                     name=f"w{l}_sb"))
                nc.sync.dma_start(w_sb[l][:], w_in[l].ap())
                gam_sb.append(PE_.tile([128, 1], F32, tag=f"g{l}", name=f"g{l}_sb"))
                nc.sync.dma_start(gam_sb[l][:], gam_in[l].ap())
                bet_sb.append(PE_.tile([128, 1], F32, tag=f"b{l}", name=f"b{l}_sb"))
                nc.sync.dma_start(bet_sb[l][:], bet_in[l].ap())
                a_sb.append(PE_.tile([128, 1], F32, tag=f"a{l}", name=f"a{l}_sb"))
                nc.sync.dma_start(a_sb[l][:], a_in[l].ap())
            zero_sb = PE_.tile([128, 128], F32, tag="zero")
            nc.vector.memset(zero_sb[:], 0.0)
            eps_sb = PE_.tile([128, 1], F32, tag="eps")
            nc.vector.memset(eps_sb[:], EPS)

            actT = PA_.tile([128, SH], F32, tag="actT")  # h as [feat, rows]

            # ---- load x, transpose into actT
            for t in range(NT):
                xt = PW.tile([128, 128], F32, tag="xt")
                nc.sync.dma_start(xt[:], x_r[t])
                tp = PT.tile([128, 128], F32, tag="tp")
                nc.tensor.transpose(out=tp[:], in_=xt[:], identity=ident[:])
                nc.vector.tensor_copy(actT[:, 128 * t : 128 * (t + 1)], tp[:])

            for l in range(2):
                # ---- v' table: vloc[t] = f16(dinv_src * (W.T @ actT)[.,t].T)
                for (o, cw) in chunks:
                    vp = PAP.tile([128, 512], F32, tag="vp")
                    nc.tensor.matmul(out=vp[:, :cw], lhsT=w_sb[l][:],
                                     rhs=actT[:, o : o + cw],
                                     start=True, stop=True)
                    vt = PW.tile([128, 512], F16, tag="vt")
                    nc.vector.tensor_copy(vt[:, :cw], vp[:, :cw])
                    for s in range(0, cw, 128):
                        t = (o + s) // 128
                        tp = PT.tile([128, 128], F16, tag="tph")
                        nc.tensor.transpose(out=tp[:], in_=vt[:, s : s + 128],
                                            identity=ident16[:])
                        vv = PW.tile([128, 128], F16, tag="vv")
                        nc.vector.tensor_scalar(
                            vv[:], tp[:], dinv_sb[:, t : t + 1], None,
                            op0=mybir.AluOpType.mult)
                        nc.sync.dma_start(vloc_r[t], vv[:])

                # ---- halo exchange: AllGather the fp16 table
                nc.gpsimd.collective_compute(
                    "AllGather", mybir.AluOpType.bypass, replica_groups=rg,
                    ins=[vloc.ap().opt()], outs=[vfull.ap().opt()])

                # ---- gather + selector-matmul scatter, pair by pair
                call_tiles = [None] * ncalls
                band_next = [band_off[j] // CALLSZ for j in range(NSLAB)]
                band_end = [band_off[j + 1] // CALLSZ for j in range(NSLAB)]

                def issue_through(P):
                    # gather everything needed for pairs <= P, per band
                    for j in range(NSLAB):
                        limit = seg_off[j][P] + 128 * K[j][P]
                        while (band_next[j] < band_end[j]
                               and band_next[j] * CALLSZ < limit):
                            k = band_next[j]
                            mt = PM.tile([128, CALLSZ // 128, 128], F16,
                                         tag="mt", name=f"mt{l}_{k}")
                            nc.gpsimd.dma_gather(
                                out_ap=mt[:],
                                in_ap=vfull.ap()[j * SLAB :
                                                 min((j + 1) * SLAB,
                                                     NB * SH), :],
                                idxs_ap=gidx_sb[:, k * CALLSZ // 16 :
                                                (k + 1) * CALLSZ // 16],
                                num_idxs=CALLSZ, num_idxs_reg=CALLSZ,
                                elem_size=D, single_packet=False,
                                queue_num=k % 4)
                            call_tiles[k] = mt
                            band_next[j] += 1

                for P in range(NP):
                    issue_through(min(P + 2, NP - 1))
                    pcw = min(PAIR, SH - P * PAIR)
                    ps = PP.tile([128, PAIR], F32, tag="ps")
                    # chunk list for this pair across bands
                    pchunks = []
                    for j in range(NSLAB):
                        g0 = seg_off[j][P] // 128
                        pchunks.append((g0, K[j][P]))
                    total = sum(k for _, k in pchunks)
                    done = 0
                    for (g0, kk) in pchunks:
                        if kk == 0:
                            continue
                        sel = PSL.tile([128, kk, PAIR], F16, tag=f"sel{kk}", name=f"sel{P}_{kk}")
                        nc.vector.tensor_tensor(
                            out=sel[:],
                            in0=iota_sb[:].unsqueeze(1).broadcast_to(
                                [128, kk, PAIR]),
                            in1=dst_sb[:, g0 : g0 + kk].unsqueeze(2)
                                .broadcast_to([128, kk, PAIR]),
                            op=mybir.AluOpType.is_equal)
                        for i in range(kk):
                            g = g0 + i
                            mt = call_tiles[g // 16]
                            nc.tensor.matmul(
                                out=ps[:, :pcw],
                                lhsT=mt[:, g % 16, :],
                                rhs=sel[:, i, :pcw],
                                start=(done == 0), stop=(done == total - 1))
                            done += 1
                    # ---- conv = dinv*(ps + dinv*v); v recomputed on PE
                    vq = PV.tile([128, PAIR], F32, tag="vq")
                    nc.tensor.matmul(out=vq[:, :pcw], lhsT=w_sb[l][:],
                                     rhs=actT[:, P * PAIR : P * PAIR + pcw],
                                     start=True, stop=True)
                    dsl = dnr_sb[:, P * PAIR : P * PAIR + pcw]
                    t1 = PW.tile([128, PAIR], F32, tag="t1")
                    nc.vector.tensor_tensor(out=t1[:, :pcw], in0=vq[:, :pcw],
                                            in1=dsl, op=mybir.AluOpType.mult)
                    t2 = PW.tile([128, PAIR], F32, tag="t2")
                    nc.vector.tensor_tensor(out=t2[:, :pcw], in0=ps[:, :pcw],
                                            in1=t1[:, :pcw],
                                            op=mybir.AluOpType.add)
                    nc.vector.tensor_tensor(
                        out=actT[:, P * PAIR : P * PAIR + pcw],
                        in0=t2[:, :pcw], in1=dsl, op=mybir.AluOpType.mult)

                # ---- BN stats (biased, over the real N rows; pad rows are 0)
                nk = len(chunks)
                sumc = PS.tile([128, nk], F32, tag="sumc")
                sqc = PS.tile([128, nk], F32, tag="sqc")
                for k, (o, cw) in enumerate(chunks):
                    nc.vector.tensor_reduce(
                        out=sumc[:, k : k + 1], in_=actT[:, o : o + cw],
                        axis=mybir.AxisListType.X, op=mybir.AluOpType.add)
                    sq = PW.tile([128, 512], F32, tag="sq")
                    nc.scalar.activation(
                        out=sq[:, :cw], in_=actT[:, o : o + cw],
                        func=mybir.ActivationFunctionType.Square,
                        bias=zero_sb[:, 0:1],
                        accum_out=sqc[:, k : k + 1])
                stats_sb = PS.tile([128, 2], F32, tag="stats")
                nc.vector.tensor_reduce(out=stats_sb[:, 0:1], in_=sumc[:],
                                        axis=mybir.AxisListType.X,
                                        op=mybir.AluOpType.add)
                nc.vector.tensor_reduce(out=stats_sb[:, 1:2], in_=sqc[:],
                                        axis=mybir.AxisListType.X,
                                        op=mybir.AluOpType.add)
                nc.sync.dma_start(stats_in.ap(), stats_sb[:])
                nc.gpsimd.collective_compute(
                    "AllReduce", mybir.AluOpType.add, replica_groups=rg,
                    ins=[stats_in.ap().opt()], outs=[stats_out.ap().opt()])
                stats2 = PS.tile([128, 2], F32, tag="stats2")
                nc.sync.dma_start(stats2[:], stats_out.ap())

                # ---- BN affine params ([128,1] each)
                mu = PS.tile([128, 1], F32, tag="mu")
                nc.vector.tensor_scalar(mu[:], stats2[:, 0:1], 1.0 / N, None,
                                        op0=mybir.AluOpType.mult)
                e2 = PS.tile([128, 1], F32, tag="e2")
                nc.vector.tensor_scalar(e2[:], stats2[:, 1:2], 1.0 / N, None,
                                        op0=mybir.AluOpType.mult)
                var = PS.tile([128, 1], F32, tag="var")
                nc.vector.scalar_tensor_tensor(
                    out=var[:], in0=mu[:], scalar=-1.0, in1=mu[:],
                    op0=mybir.AluOpType.mult, op1=mybir.AluOpType.mult)
                nc.vector.tensor_tensor(out=var[:], in0=e2[:], in1=var[:],
                                        op=mybir.AluOpType.add)
                sd = PS.tile([128, 1], F32, tag="sd")
                nc.scalar.activation(out=sd[:], in_=var[:],
                                     func=mybir.ActivationFunctionType.Sqrt,
                                     bias=eps_sb[:, 0:1])
                rinv = PS.tile([128, 1], F32, tag="rinv")
                nc.vector.reciprocal(rinv[:], sd[:])
                alpha = PS.tile([128, 1], F32, tag="alpha")
                nc.vector.tensor_tensor(out=alpha[:], in0=gam_sb[l][:],
                                        in1=rinv[:], op=mybir.AluOpType.mult)
                bias_p = PS.tile([128, 1], F32, tag="biasp")
                nc.vector.scalar_tensor_tensor(
                    out=bias_p[:], in0=alpha[:], scalar=-1.0, in1=mu[:],
                    op0=mybir.AluOpType.mult, op1=mybir.AluOpType.mult)
                nc.vector.tensor_tensor(out=bias_p[:], in0=bet_sb[l][:],
                                        in1=bias_p[:], op=mybir.AluOpType.add)
                nalpha = PS.tile([128, 1], F32, tag="nalpha")
                nc.vector.tensor_scalar(nalpha[:], alpha[:], -1.0, None,
                                        op0=mybir.AluOpType.mult)
                nbias = PS.tile([128, 1], F32, tag="nbias")
                nc.vector.tensor_scalar(nbias[:], bias_p[:], -1.0, None,
                                        op0=mybir.AluOpType.mult)
                na = PS.tile([128, 1], F32, tag="na")
                nc.vector.tensor_scalar(na[:], a_sb[l][:], -1.0, None,
                                        op0=mybir.AluOpType.mult)

                # ---- fused BN + PReLU: y = relu(z) - a*relu(-z)
                for (o, cw) in chunks:
                    pos = PW.tile([128, 512], F32, tag="pos")
                    nc.scalar.activation(
                        out=pos[:, :cw], in_=actT[:, o : o + cw],
                        func=mybir.ActivationFunctionType.Relu,
                        bias=bias_p[:, :1], scale=alpha[:, :1])
                    neg = PW.tile([128, 512], F32, tag="neg")
                    nc.scalar.activation(
                        out=neg[:, :cw], in_=actT[:, o : o + cw],
                        func=mybir.ActivationFunctionType.Relu,
                        bias=nbias[:, :1], scale=nalpha[:, :1])
                    nc.vector.scalar_tensor_tensor(
                        out=actT[:, o : o + cw], in0=neg[:, :cw],
                        scalar=na[:, :1], in1=pos[:, :cw],
                        op0=mybir.AluOpType.mult, op1=mybir.AluOpType.add)

            # ---- write h2 back as [rows, feat]
            for t in range(NT):
                tp = PT.tile([128, 128], F32, tag="tp")
                nc.tensor.transpose(out=tp[:],
                                    in_=actT[:, 128 * t : 128 * (t + 1)],
                                    identity=ident[:])
                ot = PW.tile([128, 128], F32, tag="ot")
                nc.vector.tensor_copy(ot[:], tp[:])
                nc.sync.dma_start(out_r[t], ot[:])

    nc.compile()
    return nc


# ------------------------------------------------------------------- driver

_CACHE: dict = {}


def _get_compiled(key, N, SH, S, K, band_off, seg_off):
    if key not in _CACHE:
        nc = build_kernel(N, SH, S, K, band_off, seg_off)
        nc.m = get_hw_module(nc.m)
        _CACHE[key] = nc
    return _CACHE[key]


def make_in_maps(pre, w0, b0, gamma0, beta0, a0, w1, b1, gamma1, beta1, a1):
    def col(v):
        return np.ascontiguousarray(np.asarray(v, np.float32).reshape(-1, 1))

    def rep(v):
        return np.full((128, 1), np.float32(np.asarray(v).reshape(-1)[0]),
                       np.float32)

    iota = np.tile(np.arange(PAIR, dtype=np.float16)[None, :], (128, 1))
    maps = []
    for c in range(NB):
        maps.append({
            "x": pre["x_sh"][c],
            "gidx": pre["gidx"][c],
            "dst16": pre["dst16"][c],
            "dinvrow": pre["dinvrow"][c],
            "dinv_cols": pre["dinv_cols"][c],
            "iota256": iota,
            "w0": np.ascontiguousarray(np.asarray(w0, np.float32)),
            "w1": np.ascontiguousarray(np.asarray(w1, np.float32)),
            "gamma0": col(gamma0), "beta0": col(beta0), "a0": rep(a0),
            "gamma1": col(gamma1), "beta1": col(beta1), "a1": rep(a1),
        })
    return maps


def kernel(x, edge_index, w0, b0, gamma0, beta0, a0,
           w1, b1, gamma1, beta1, a1, _trace=False):
    x = np.asarray(x, np.float32)
    edge_index = np.asarray(edge_index, np.int64)
    pre = preprocess(x, edge_index)
    N, nsh, SH, S = pre["N"], pre["nsh"], pre["SH"], pre["S"]
    key = (N, SH, S, pre["K"], pre["band_off"])
    nc = _get_compiled(key, N, SH, S, pre["K"], pre["band_off"],
                       pre["seg_off"])
    in_maps = make_in_maps(pre, w0, b0, gamma0, beta0, a0,
                           w1, b1, gamma1, beta1, a1)
    res = bass_utils.run_bass_kernel_spmd(
        nc, in_maps, core_ids=list(range(NB)), trace=_trace)
    out = np.concatenate([res.results[c]["out"][:nsh] for c in range(NB)],
                         axis=0)[:N]
    if _trace:
        kernel.last_results = res
    return np.ascontiguousarray(out)
